# revision 1
# baseline (speedup 1.0000x reference)
"""Trainium2 Bass kernel for nn_Net_67954972557347 (dense_mlp).

Network: a1 = lrelu(a@Wa+ba) [B,68]; b1 = lrelu(b@Wb+bb) [B,68];
c = [a1|b1|meta] [B,140]; then 10 lrelu'd dense layers
(140->34->34->20->20->20->20->20->5->2->1), lrelu slope 0.01.

Strategy: pure data parallel over 8 cores (32768 rows each). On-device,
activations are feature-major ([feat, batch]) so each layer is a PE matmul
with the batch streaming as the moving operand (float32r / tf32 datapath,
fp32 PSUM accumulation). The host pre-transposes and packs the inputs:

  t1 [128, 32768]: rows 0:45 = a.T, 45:49 = ilrelu(meta.T),
                   rows 49:128 = b.T[0:79]
  t2 [128, 8192]:  per chunk c (512 cols): 32-row block c%4 of column
                   group c//4 = [b.T[79:102]; ones] (24 rows)

Every matmul reads a base-0 partition window of one SBUF tile with
zero-padded weight columns/rows (this walrus build only supports
tile_position (0,0)-style full windows reliably), and every matmul output
starts at PSUM partition 0. meta rides through the first-layer matmul as a
passthrough output (host pre-applies inverse-lrelu so the drain's
leaky-relu recovers it exactly); the ones row in t2 folds the b1 bias into
the B matmul.

A 10-step software pipeline processes one 512-column chunk per step with
4 PSUM banks (chunk ages at step t):
  alpha [0:72]  = [a1(68); meta(4)]              of chunk t     (1 MM)
  beta  [0:68]  = b1                             of chunk t     (2 MMs)
  dA    [0:68]  = [c0(t-1); c1(t-2)]                            (3 MMs)
  dB    [0:108] = [y(t-10); c2(t-3); c3(t-4); c4(t-5); c5(t-6);
                   c6(t-7); c7(t-8); c8(t-9)]                   (2 MMs)
The whole deep tail (c2->...->c8->y) advances one stage per step inside a
single matmul (D2) whose block-structured weights read the previous step's
dB drain. alpha/dA/dB are drained by one ACT Prelu each (per-partition
bias vector, alpha=0.01); beta drains on DVE (copy + max(0.01x, x), bias
pre-folded via the ones row).
"""

import os
import sys

import numpy as np

for _p in ("/opt/trn_rl_repo", "/root/.axon_site/_ro/trn_rl_repo"):
    if os.path.isdir(_p) and _p not in sys.path:
        sys.path.append(_p)

import concourse.bass as bass
import concourse.mybir as mybir
import concourse.tile as tile
from concourse import bacc
from concourse.bass_utils import run_bass_kernel_spmd
from bass_rust import add_dep_helper

F32 = mybir.dt.float32
F32R = mybir.dt.float32r
ALU = mybir.AluOpType
PRELU = mybir.ActivationFunctionType.Prelu

B_FULL = 262144
N_CORES = 8
B_CORE = B_FULL // N_CORES          # 32768
N = 512                              # columns per chunk (PSUM bank / fp32 cap)
PIPE = 10                            # pipeline depth in steps
ALPHA = 0.01                         # leaky-relu slope

# weight-tile column spans (B2 has 4 spans of 68 at CB2 + 68*k)
CA1, CB1, CB2 = 0, 72, 140
CL0A, CL0B, CL1 = 412, 480, 514
CD1, CD2 = 582, 690
WT_COLS = 1024
M_AL, M_BE, M_DA, M_DB = 72, 68, 68, 108


def _ilrelu(x):
    """Inverse of leaky-relu (slope 0.01)."""
    return np.where(x > 0, x, x * (1.0 / ALPHA)).astype(np.float32)


def _pack_weights(Wa, ba, Wb, bb, Ws, Bs):
    """Build the [128, WT_COLS] packed weight tile and [128, 3] bias tile."""
    W0, W1, W2, W3, W4, W5, W6, W7, W8, W9 = Ws
    B0, B1, B2, B3, B4, B5, B6, B7, B8, B9 = Bs
    wt = np.zeros((128, WT_COLS), np.float32)
    # A1: rhs t1[0:49]: rows 0:45 = a.T -> a1 (cols 0:68);
    # rows 45:49 = meta passthrough (cols 68:72)
    wt[0:45, CA1:CA1 + 68] = Wa
    wt[45:49, CA1 + 68:CA1 + 72] = np.eye(4, dtype=np.float32)
    # B1: rhs t1[0:128]: rows 49:128 = b.T[0:79]
    wt[49:128, CB1:CB1 + 68] = Wb[0:79]
    # B2 (span per k): rhs t2[0:128]: block k rows 32k:32k+23 = b.T[79:102],
    # row 32k+23 = ones -> bias bb
    for kk in range(4):
        r = 32 * kk
        c = CB2 + 68 * kk
        wt[r:r + 23, c:c + 68] = Wb[79:102]
        wt[r + 23, c:c + 68] = bb
    # L0a: rhs a1t[0:72]: a1 -> W0[0:68], meta -> W0[136:140]; c0 cols 0:34
    wt[0:68, CL0A:CL0A + 34] = W0[0:68]
    wt[68:72, CL0A:CL0A + 34] = W0[136:140]
    # L0b: rhs b1t[0:68] -> c0 (cols 0:34)
    wt[0:68, CL0B:CL0B + 34] = W0[68:136]
    # L1: rhs DA[0:34] = c0 -> c1 (cols 34:68)
    wt[0:34, CL1 + 34:CL1 + 68] = W1
    # D1: rhs DA[0:68]: rows 34:68 = c1 -> c2 (cols 1:21)
    wt[34:68, CD1 + 1:CD1 + 21] = W2
    # D2: rhs DB[0:108]: the whole tail chain advances one stage
    wt[1:21, CD2 + 21:CD2 + 41] = W3      # c2 -> c3
    wt[21:41, CD2 + 41:CD2 + 61] = W4     # c3 -> c4
    wt[41:61, CD2 + 61:CD2 + 81] = W5     # c4 -> c5
    wt[61:81, CD2 + 81:CD2 + 101] = W6    # c5 -> c6
    wt[81:101, CD2 + 101:CD2 + 106] = W7  # c6 -> c7
    wt[101:106, CD2 + 106:CD2 + 108] = W8  # c7 -> c8
    wt[106:108, CD2:CD2 + 1] = W9         # c8 -> y

    bias = np.zeros((128, 3), np.float32)
    bias[0:68, 0] = ba                    # alpha bank
    bias[0:34, 1] = B0                    # dA bank
    bias[34:68, 1] = B1
    bias[0:1, 2] = B9                     # dB bank
    bias[1:21, 2] = B2
    bias[21:41, 2] = B3
    bias[41:61, 2] = B4
    bias[61:81, 2] = B5
    bias[81:101, 2] = B6
    bias[101:106, 2] = B7
    bias[106:108, 2] = B8
    return wt, bias


def _pack_core_inputs(a, b, meta, n_chunks):
    """Pack one core's shard into the t1/t2 DMA streams."""
    bc = n_chunks * N
    t1 = np.empty((128, bc), np.float32)
    t1[0:45] = a[:bc].T
    t1[45:49] = _ilrelu(meta[:bc].T)
    t1[49:128] = b[:bc, 0:79].T
    n_super = (n_chunks + 3) // 4
    t2 = np.zeros((128, n_super * N), np.float32)
    bT_tail = np.ascontiguousarray(b[:bc, 79:102].T)
    for c in range(n_chunks):
        r = 32 * (c % 4)
        cs = slice(c * N, (c + 1) * N)
        ds = slice((c // 4) * N, (c // 4 + 1) * N)
        t2[r:r + 23, ds] = bT_tail[:, cs]
        t2[r + 23, ds] = 1.0
    return t1, t2


def build_bass(n_chunks):
    """Build + compile the per-core Bass program (same on all 8 cores)."""
    nc = bacc.Bacc(None, target_bir_lowering=False, debug=False)
    n_steps = n_chunks + PIPE
    n_super = (n_chunks + 3) // 4

    t1_d = nc.dram_tensor("t1", [128, n_chunks * N], F32,
                          kind="ExternalInput")
    t2_d = nc.dram_tensor("t2", [128, n_super * N], F32,
                          kind="ExternalInput")
    wt_d = nc.dram_tensor("wt", [128, WT_COLS], F32, kind="ExternalInput")
    bias_d = nc.dram_tensor("bias", [128, 3], F32, kind="ExternalInput")
    y_d = nc.dram_tensor("y", [1, n_chunks * N], F32, kind="ExternalOutput")

    with tile.TileContext(nc) as tc:
        with (
            tc.tile_pool(name="const", bufs=1) as constp,
            tc.tile_pool(name="t1p", bufs=3) as t1p,
            tc.tile_pool(name="t2p", bufs=2) as t2p,
            tc.tile_pool(name="actp", bufs=3) as actp,
            tc.tile_pool(name="dp", bufs=2) as dp,
            tc.tile_pool(name="ps", bufs=2, space=bass.MemorySpace.PSUM) as ps,
        ):
            wt = constp.tile([128, WT_COLS], F32R, tag="wt")
            bias = constp.tile([128, 3], F32, tag="bias")
            z1 = constp.tile([128, N], F32R, tag="z1")
            nc.sync.dma_start(wt[:], wt_d[:].bitcast(F32R))
            nc.sync.dma_start(bias[:], bias_d[:])
            nc.gpsimd.memset(z1[:].bitcast(F32), 0.0)

            def w(c0, c1):
                return wt[:, c0:c1]

            def chain(*insts):
                for i in range(1, len(insts)):
                    add_dep_helper(insts[i].ins, insts[i - 1].ins,
                                   sync=False, reason="psum acc order")

            t1s, t2s, a1s, b1s, das, dbs = {}, {}, {}, {}, {}, {}
            for d, pool, tag in ((a1s, actp, "a1"), (b1s, actp, "b1"),
                                 (das, dp, "da"), (dbs, dp, "db")):
                d[-1] = pool.tile([128, N], F32R, tag=tag, name=f"{tag}_zm1")
                nc.gpsimd.memset(d[-1][:].bitcast(F32), 0.0)

            for t in range(n_steps):
                # ---- DMAs in ----
                if t < n_chunks:
                    t1s[t] = t1p.tile([128, N], F32R, tag="t1",
                                      name=f"t1_{t}")
                    nc.sync.dma_start(
                        t1s[t][:], t1_d[:, t * N:(t + 1) * N].bitcast(F32R))
                    if t % 4 == 0:
                        s = t // 4
                        t2s[s] = t2p.tile([128, N], F32R, tag="t2",
                                          name=f"t2_{s}")
                        nc.sync.dma_start(
                            t2s[s][:],
                            t2_d[:, s * N:(s + 1) * N].bitcast(F32R))

                mm = nc.tensor.matmul
                k = t % 4
                rhs1 = t1s[t][:] if t < n_chunks else z1[:]

                al = ps.tile([128, N], F32, tag="al", name=f"al_{t}")
                be = ps.tile([128, N], F32, tag="be", name=f"be_{t}")
                dA = ps.tile([128, N], F32, tag="dA", name=f"dA_{t}")
                dB = ps.tile([128, N], F32, tag="dB", name=f"dB_{t}")

                # ---- alpha: A1 (a1 + meta passthrough) ----
                mm(al[0:M_AL], w(CA1, CA1 + M_AL)[0:49], rhs1[0:49],
                   start=True, stop=True, tile_position=(0, 0))

                # ---- beta: B1 + B2 (b-tail + ones->bias) ----
                i1 = mm(be[0:M_BE], w(CB1, CB1 + M_BE)[0:128], rhs1,
                        start=True, stop=t >= n_chunks,
                        tile_position=(0, 0))
                if t < n_chunks:
                    cb2 = CB2 + 68 * k
                    i2 = mm(be[0:M_BE], wt[0:128, cb2:cb2 + M_BE],
                            t2s[t // 4][0:128],
                            start=False, stop=True, tile_position=(0, 0))
                    chain(i1, i2)

                # ---- dA: L0a + L0b (c0), L1 (c1) ----
                i1 = mm(dA[0:M_DA], w(CL0A, CL0A + M_DA)[0:M_AL],
                        a1s[t - 1][0:M_AL],
                        start=True, stop=False, tile_position=(0, 0))
                i2 = mm(dA[0:34], w(CL0B, CL0B + 34)[0:M_BE],
                        b1s[t - 1][0:M_BE],
                        start=False, stop=False, tile_position=(0, 0))
                i3 = mm(dA[0:M_DA], w(CL1, CL1 + M_DA)[0:34],
                        das[t - 1][0:34],
                        start=False, stop=True, tile_position=(0, 0))
                chain(i1, i2, i3)

                # ---- dB: D1 (c2), D2 (tail chain c3..c8, y) ----
                i1 = mm(dB[0:M_DB], w(CD1, CD1 + M_DB)[0:M_DA],
                        das[t - 1][0:M_DA],
                        start=True, stop=False, tile_position=(0, 0))
                i2 = mm(dB[0:M_DB], w(CD2, CD2 + M_DB)[0:M_DB],
                        dbs[t - 1][0:M_DB],
                        start=False, stop=True, tile_position=(0, 0))
                chain(i1, i2)

                # ---- drains ----
                a1s[t] = actp.tile([128, N], F32R, tag="a1", name=f"a1_{t}")
                nc.scalar.activation(a1s[t][0:M_AL], al[0:M_AL], PRELU,
                                     bias=bias[0:M_AL, 0:1], alpha=ALPHA)
                das[t] = dp.tile([128, N], F32R, tag="da", name=f"da_{t}")
                nc.scalar.activation(das[t][0:M_DA], dA[0:M_DA], PRELU,
                                     bias=bias[0:M_DA, 1:2], alpha=ALPHA)
                dbs[t] = dp.tile([128, N], F32R, tag="db", name=f"db_{t}")
                nc.scalar.activation(dbs[t][0:M_DB], dB[0:M_DB], PRELU,
                                     bias=bias[0:M_DB, 2:3], alpha=ALPHA)
                b1s[t] = actp.tile([128, N], F32R, tag="b1", name=f"b1_{t}")
                nc.vector.tensor_copy(b1s[t][0:M_BE], be[0:M_BE])
                nc.vector.scalar_tensor_tensor(
                    b1s[t][0:M_BE], b1s[t][0:M_BE], ALPHA, b1s[t][0:M_BE],
                    ALU.mult, ALU.max)

                # ---- y out ----
                if t >= PIPE:
                    c = t - PIPE
                    nc.gpsimd.dma_start(
                        y_d[:, c * N:(c + 1) * N].bitcast(F32R),
                        dbs[t][0:1])

    nc.compile()
    return nc


_NC_CACHE = {}


def _get_nc(n_chunks):
    if n_chunks not in _NC_CACHE:
        _NC_CACHE[n_chunks] = build_bass(n_chunks)
    return _NC_CACHE[n_chunks]


def run_cores(inputs, n_chunks, cores, trace=False, trace_kwargs=None):
    """Pack inputs, run the SPMD kernel on the given cores, return
    (per-core y arrays, BassKernelResults)."""
    a = np.asarray(inputs["a"], np.float32)
    b = np.asarray(inputs["b"], np.float32)
    meta = np.asarray(inputs["meta"], np.float32)
    Ws = [np.asarray(inputs[f"W{i}"], np.float32) for i in range(10)]
    Bs = [np.asarray(inputs[f"B{i}"], np.float32) for i in range(10)]
    wt, bias = _pack_weights(np.asarray(inputs["Wa"], np.float32),
                             np.asarray(inputs["ba"], np.float32),
                             np.asarray(inputs["Wb"], np.float32),
                             np.asarray(inputs["bb"], np.float32), Ws, Bs)
    in_maps = []
    for r in cores:
        sl = slice(r * B_CORE, r * B_CORE + n_chunks * N)
        t1, t2 = _pack_core_inputs(a[sl], b[sl], meta[sl], n_chunks)
        in_maps.append({"t1": t1, "t2": t2, "wt": wt, "bias": bias})
    nc = _get_nc(n_chunks)
    kw = dict(trace=trace)
    if trace_kwargs:
        kw.update(trace_kwargs)
    res = run_bass_kernel_spmd(nc, in_maps, list(range(len(cores))), **kw)
    return [res.results[i]["y"] for i in range(len(cores))], res


def kernel(**inputs):
    n_chunks = B_CORE // N
    ys, _ = run_cores(inputs, n_chunks, list(range(N_CORES)))
    out = np.empty((B_FULL, 1), np.float32)
    for r in range(N_CORES):
        out[r * B_CORE:(r + 1) * B_CORE, 0] = ys[r][0]
    return out



# revision 7
# speedup vs baseline: 1.1244x; 1.1244x over previous
"""Trainium2 Bass kernel for nn_Net_67954972557347 (dense_mlp).

Network: a1 = lrelu(a@Wa+ba) [B,68]; b1 = lrelu(b@Wb+bb) [B,68];
c = [a1|b1|meta] [B,140]; then 10 lrelu'd dense layers
(140->34->34->20->20->20->20->20->5->2->1), lrelu slope 0.01.

Strategy: pure data parallel over 8 cores (32768 rows each), activations
feature-major ([feat, batch]); batch streams 512 columns per pipeline
step through the PE (fp32r datapath).

v2 design — 6 matmuls / 2 PSUM tiles / 2 drain groups per step:
  psT (3 banks, 1536 cols):
    cols 0:512   bankE [c0; c2; c4; c6; c8; ones] <- MM3(T1h) + MM4(T2)
                                                     + MM5(TOh)
    cols 512:1024 bankO [c1; c3; c5; c7; y; ones] <- MM6(TEh)
    cols 1024:1536 bank1 [a1; meta; ones]         <- MM1(t1)
  ps2 (1 bank): bank2 [b1; ones]                  <- MM2(t2)
t1 = [a.T; ilrelu(meta); ones], t2 = [b.T; ones] are the DMA streams.
The even/odd chain banks advance all ten tail layers in two
block-diagonal matmuls. ALL biases are folded in-PSUM via ones-row
passthrough columns, so drains are pure leaky-relu: psT drains in ONE
1536-col ACT Prelu into the TEO1 tile (whose three 512-col halves are
the next step's matmul rhs windows); ps2 drains on DVE (copy +
max(0.01x, x), PSUM cannot be a dual stt operand).

Latency hiding: matmuls read tiles drained TWO steps ago (age-2) and
input DMA is prefetched two steps ahead, so the PE's in-order queue
never waits on same-step drains and the tensor engine stays
back-to-back busy (keeps the PE p-state clock ramped). Pipeline depth
2 steps/layer * 10 stages = 20 steps.
"""

import os
import sys

import numpy as np

for _p in ("/opt/trn_rl_repo", "/root/.axon_site/_ro/trn_rl_repo"):
    if os.path.isdir(_p) and _p not in sys.path:
        sys.path.append(_p)

import concourse.bass as bass
import concourse.mybir as mybir
import concourse.tile as tile
from concourse import bacc
from concourse.bass_utils import run_bass_kernel_spmd
from bass_rust import add_dep_helper

F32 = mybir.dt.float32
F32R = mybir.dt.float32r
ALU = mybir.AluOpType
PRELU = mybir.ActivationFunctionType.Prelu

B_FULL = 262144
N_CORES = 8
B_CORE = B_FULL // N_CORES          # 32768
N = 512                              # columns per chunk (fp32 PSUM bank)
PIPE = 20                            # 10 stages x 2-step latency
AGE = 2                              # drain-to-consume latency in steps
ALPHA = 0.01                         # leaky-relu slope

# partition row counts
K1 = 50          # t1: a(45) + ilrelu(meta)(4) + ones(1)
K2 = 103         # t2: b(102) + ones(1)
M2 = 69          # bank2 drain: b1(68) + ones(1)
MT = 97          # psT drain partitions (bankE exact; bankO/bank1 padded)

# weight tile column spans
CM1, CM2, CM3, CM4, CM5, CM6 = 0, 97, 166, 263, 297, 394
WT_COLS = 512


def _ilrelu(x):
    """Inverse of leaky-relu (slope 0.01)."""
    return np.where(x > 0, x, x * (1.0 / ALPHA)).astype(np.float32)


def _pack_weights(Wa, ba, Wb, bb, Ws, Bs):
    """Build the [128, WT_COLS] packed weight tile (biases via ones rows)."""
    W0, W1, W2, W3, W4, W5, W6, W7, W8, W9 = Ws
    B0, B1, B2, B3, B4, B5, B6, B7, B8, B9 = Bs
    wt = np.zeros((128, WT_COLS), np.float32)
    # MM1: rhs t1[0:50] -> bank1 [a1(0:68); meta(68:72); ones(72)]
    c = CM1
    wt[0:45, c:c + 68] = Wa
    wt[45:49, c + 68:c + 72] = np.eye(4, dtype=np.float32)
    wt[49, c:c + 68] = ba
    wt[49, c + 72] = 1.0
    # MM2: rhs t2[0:103] -> bank2 [b1(0:68); ones(68)]
    c = CM2
    wt[0:102, c:c + 68] = Wb
    wt[102, c:c + 68] = bb
    wt[102, c + 68] = 1.0
    # MM3: rhs T1h[0:73] -> bankE c0 part (cols 0:34) + ones (col 96)
    c = CM3
    wt[0:68, c:c + 34] = W0[0:68]
    wt[68:72, c:c + 34] = W0[136:140]
    wt[72, c:c + 34] = B0
    wt[72, c + 96] = 1.0
    # MM4: rhs T2[0:69] -> bankE c0 part (cols 0:34)
    c = CM4
    wt[0:68, c:c + 34] = W0[68:136]
    # MM5: rhs TOh[0:81] = [c1;c3;c5;c7;y;ones] -> bankE evens
    c = CM5
    wt[0:34, c + 34:c + 54] = W2    # c1 -> c2
    wt[34:54, c + 54:c + 74] = W4   # c3 -> c4
    wt[54:74, c + 74:c + 94] = W6   # c5 -> c6
    wt[74:79, c + 94:c + 96] = W8   # c7 -> c8
    wt[80, c + 34:c + 54] = B2
    wt[80, c + 54:c + 74] = B4
    wt[80, c + 74:c + 94] = B6
    wt[80, c + 94:c + 96] = B8
    # MM6: rhs TEh[0:97] = [c0;c2;c4;c6;c8;ones] -> bankO odds
    c = CM6
    wt[0:34, c:c + 34] = W1         # c0 -> c1
    wt[34:54, c + 34:c + 54] = W3   # c2 -> c3
    wt[54:74, c + 54:c + 74] = W5   # c4 -> c5
    wt[74:94, c + 74:c + 79] = W7   # c6 -> c7
    wt[94:96, c + 79:c + 80] = W9   # c8 -> y
    wt[96, c:c + 34] = B1
    wt[96, c + 34:c + 54] = B3
    wt[96, c + 54:c + 74] = B5
    wt[96, c + 74:c + 79] = B7
    wt[96, c + 79] = B9[0]
    wt[96, c + 80] = 1.0
    return wt


def _pack_core_inputs(a, b, meta, n_chunks):
    """Pack one core's shard into the t1/t2 DMA streams."""
    bc = n_chunks * N
    t1 = np.empty((K1, bc), np.float32)
    t1[0:45] = a[:bc].T
    t1[45:49] = _ilrelu(meta[:bc].T)
    t1[49] = 1.0
    t2 = np.empty((K2, bc), np.float32)
    t2[0:102] = b[:bc].T
    t2[102] = 1.0
    return t1, t2


def build_bass(n_chunks):
    """Build + compile the per-core Bass program (same on all 8 cores)."""
    nc = bacc.Bacc(None, target_bir_lowering=False, debug=False)
    n_steps = n_chunks + PIPE

    t1_d = nc.dram_tensor("t1", [K1, n_chunks * N], F32, kind="ExternalInput")
    t2_d = nc.dram_tensor("t2", [K2, n_chunks * N], F32, kind="ExternalInput")
    wt_d = nc.dram_tensor("wt", [128, WT_COLS], F32, kind="ExternalInput")
    y_d = nc.dram_tensor("y", [1, n_chunks * N], F32, kind="ExternalOutput")

    with tile.TileContext(nc) as tc:
        with (
            tc.tile_pool(name="const", bufs=1) as constp,
            tc.tile_pool(name="t1p", bufs=4) as t1p,
            tc.tile_pool(name="t2p", bufs=4) as t2p,
            tc.tile_pool(name="teop", bufs=3) as teop,
            tc.tile_pool(name="bp", bufs=3) as bp,
            tc.tile_pool(name="psE", bufs=2, space=bass.MemorySpace.PSUM) as psEp,
            tc.tile_pool(name="psO", bufs=2, space=bass.MemorySpace.PSUM) as psOp,
            tc.tile_pool(name="ps1", bufs=2, space=bass.MemorySpace.PSUM) as ps1p,
            tc.tile_pool(name="ps2", bufs=2, space=bass.MemorySpace.PSUM) as ps2p,
        ):
            wt = constp.tile([128, WT_COLS], F32R, tag="wt")
            z1 = constp.tile([128, N], F32R, tag="z1")
            nc.sync.dma_start(wt[:], wt_d[:].bitcast(F32R))
            nc.gpsimd.memset(z1[:].bitcast(F32), 0.0)

            def w(c0, c1):
                return wt[:, c0:c1]

            def chain(*insts):
                for i in range(1, len(insts)):
                    add_dep_helper(insts[i].ins, insts[i - 1].ins,
                                   sync=False, reason="psum acc order")

            t1s, t2s, teos, b1s = {}, {}, {}, {}
            for age in (-1, -2):
                teos[age] = teop.tile([128, 3 * N], F32R, tag="teo",
                                      name=f"teo_z{age}")
                nc.gpsimd.memset(teos[age][:].bitcast(F32), 0.0)
                b1s[age] = bp.tile([128, N], F32R, tag="b1",
                                   name=f"b1_z{age}")
                nc.gpsimd.memset(b1s[age][:].bitcast(F32), 0.0)

            def dma_in(c):
                if c < n_chunks:
                    t1s[c] = t1p.tile([128, N], F32R, tag="t1", name=f"t1_{c}")
                    nc.sync.dma_start(
                        t1s[c][0:K1], t1_d[:, c * N:(c + 1) * N].bitcast(F32R))
                    t2s[c] = t2p.tile([128, N], F32R, tag="t2", name=f"t2_{c}")
                    nc.sync.dma_start(
                        t2s[c][0:K2], t2_d[:, c * N:(c + 1) * N].bitcast(F32R))

            dma_in(0)
            dma_in(1)

            for t in range(n_steps):
                dma_in(t + AGE)
                mm = nc.tensor.matmul

                teo = teos[t - AGE]
                b1 = b1s[t - AGE]
                real = t < n_chunks
                rhs1 = t1s[t][0:K1] if real else z1[0:K1]
                rhs2 = t2s[t][0:K2] if real else z1[0:K2]

                psE = psEp.tile([128, N], F32, tag="psE", name=f"psE_{t}")
                psO = psOp.tile([128, N], F32, tag="psO", name=f"psO_{t}")
                ps1 = ps1p.tile([128, N], F32, tag="ps1", name=f"ps1_{t}")
                ps2 = ps2p.tile([128, N], F32, tag="ps2", name=f"ps2_{t}")

                # ---- chain banks first: their deps are 2 steps old ----
                i1 = mm(psE[0:MT], w(CM3, CM3 + MT)[0:73],
                        teo[0:73, 2 * N:3 * N],
                        start=True, stop=False, tile_position=(0, 0))
                i2 = mm(psE[0:34], w(CM4, CM4 + 34)[0:M2], b1[0:M2],
                        start=False, stop=False, tile_position=(0, 0))
                i3 = mm(psE[0:MT], w(CM5, CM5 + MT)[0:81],
                        teo[0:81, N:2 * N],
                        start=False, stop=True, tile_position=(0, 0))
                chain(i1, i2, i3)

                mm(psO[0:MT], w(CM6, CM6 + MT)[0:MT],
                   teo[0:MT, 0:N],
                   start=True, stop=True, tile_position=(0, 0))

                # ---- stage 1 ----
                mm(ps1[0:MT], w(CM1, CM1 + MT)[0:K1], rhs1,
                   start=True, stop=True, tile_position=(0, 0))
                mm(ps2[0:M2], w(CM2, CM2 + M2)[0:K2], rhs2,
                   start=True, stop=True, tile_position=(0, 0))

                # ---- drains (pure lrelu; biases already in PSUM) ----
                teos[t] = teop.tile([128, 3 * N], F32R, tag="teo",
                                    name=f"teo_{t}")
                nc.scalar.activation(teos[t][0:MT, 0:N], psE[0:MT],
                                     PRELU, alpha=ALPHA)
                nc.scalar.activation(teos[t][0:MT, N:2 * N], psO[0:MT],
                                     PRELU, alpha=ALPHA)
                nc.scalar.activation(teos[t][0:MT, 2 * N:3 * N], ps1[0:MT],
                                     PRELU, alpha=ALPHA)
                b1s[t] = bp.tile([128, N], F32R, tag="b1", name=f"b1_{t}")
                nc.vector.tensor_copy(b1s[t][0:M2], ps2[0:M2])
                nc.vector.scalar_tensor_tensor(
                    b1s[t][0:M2], b1s[t][0:M2], ALPHA, b1s[t][0:M2],
                    ALU.mult, ALU.max)

                # ---- y out (row 79 of the odd half) ----
                if t >= PIPE:
                    c = t - PIPE
                    nc.gpsimd.dma_start(
                        y_d[:, c * N:(c + 1) * N].bitcast(F32R),
                        teos[t][79:80, N:2 * N])

    nc.compile()
    return nc


_NC_CACHE = {}


def _get_nc(n_chunks):
    if n_chunks not in _NC_CACHE:
        _NC_CACHE[n_chunks] = build_bass(n_chunks)
    return _NC_CACHE[n_chunks]


def run_cores(inputs, n_chunks, cores, trace=False, trace_kwargs=None):
    """Pack inputs, run the SPMD kernel on the given cores, return
    (per-core y arrays, BassKernelResults)."""
    a = np.asarray(inputs["a"], np.float32)
    b = np.asarray(inputs["b"], np.float32)
    meta = np.asarray(inputs["meta"], np.float32)
    Ws = [np.asarray(inputs[f"W{i}"], np.float32) for i in range(10)]
    Bs = [np.asarray(inputs[f"B{i}"], np.float32) for i in range(10)]
    wt = _pack_weights(np.asarray(inputs["Wa"], np.float32),
                       np.asarray(inputs["ba"], np.float32),
                       np.asarray(inputs["Wb"], np.float32),
                       np.asarray(inputs["bb"], np.float32), Ws, Bs)
    in_maps = []
    for r in cores:
        sl = slice(r * B_CORE, r * B_CORE + n_chunks * N)
        t1, t2 = _pack_core_inputs(a[sl], b[sl], meta[sl], n_chunks)
        in_maps.append({"t1": t1, "t2": t2, "wt": wt})
    nc = _get_nc(n_chunks)
    kw = dict(trace=trace)
    if trace_kwargs:
        kw.update(trace_kwargs)
    res = run_bass_kernel_spmd(nc, in_maps, list(range(len(cores))), **kw)
    return [res.results[i]["y"] for i in range(len(cores))], res


def kernel(**inputs):
    n_chunks = B_CORE // N
    ys, _ = run_cores(inputs, n_chunks, list(range(N_CORES)))
    out = np.empty((B_FULL, 1), np.float32)
    for r in range(N_CORES):
        out[r * B_CORE:(r + 1) * B_CORE, 0] = ys[r][0]
    return out


# revision 9
# speedup vs baseline: 1.7938x; 1.5953x over previous
"""Trainium2 Bass kernel for nn_Net_67954972557347 (dense_mlp).

Network: a1 = lrelu(a@Wa+ba) [B,68]; b1 = lrelu(b@Wb+bb) [B,68];
c = [a1|b1|meta] [B,140]; then 10 lrelu'd dense layers
(140->34->34->20->20->20->20->20->5->2->1), lrelu slope 0.01.

Strategy: pure data parallel over 8 cores (32768 rows each), activations
feature-major ([feat, batch]); batch streams 512 columns per pipeline
step through the PE (fp32r datapath).

6 matmuls / 2 PSUM tiles / 2 drain groups per step:
  psT (3 banks, 1536 cols):
    cols 0:512    bankE [c0; c2; c4; c6; c8; ones] <- MM3(T1h) + MM4(B1)
                                                      + MM5(TOh)
    cols 512:1024 bankO [c1; c3; c5; c7; y; ones]  <- MM6(TEh)
    cols 1024:1536 bank1 [a1; meta; ones]          <- MM1(t1)
  ps2 (1 bank): bank2 [b1; ones]                   <- MM2(t2)
t1 = [a.T; ilrelu(meta); ones], t2 = [b.T; ones] are the DMA streams.
The even/odd chain banks advance all ten tail layers in two
block-diagonal matmuls. ALL biases are folded in-PSUM via ones-row
passthrough columns, so drains are pure leaky-relu: psT drains in ONE
1536-col ACT Prelu into the TEO tile (whose three 512-col halves are
the next step's matmul rhs windows); ps2 drains on DVE (copy +
max(0.01x, x); PSUM cannot be a dual stt operand).

EVERY matmul uses K=128 (full-partition rhs window, zero weight rows
beyond the real contraction) and M>=69: the PE array reconfigures its
tile geometry whenever round-up(K)/round-up(M) changes between
consecutive matmuls, which locks the clock at the mid p-state (0.83
ns/col instead of 0.42 — measured 427 vs 229 ns per 512-col matmul).
Uniform 128x128 tiles keep it at full speed for free (cost scales with
the moving dim only). SBUF operand buffers are fixed, self-managed
rings, fully memset once so the padded partition rows multiply as 0.0
(never NaN garbage).

Latency hiding: matmuls read tiles drained TWO steps ago (age-2) and
input DMA is prefetched two steps ahead, so the PE's in-order queue
never waits on same-step drains. The t2 stream is padded to an even
partition count: odd-partition DMAs land on a single DMA queue instead
of spreading across all 16. Pipeline depth 2 steps/layer * 10 stages
= 20 steps.
"""

import os
import sys

import numpy as np

for _p in ("/opt/trn_rl_repo", "/root/.axon_site/_ro/trn_rl_repo"):
    if os.path.isdir(_p) and _p not in sys.path:
        sys.path.append(_p)

import concourse.bass as bass
import concourse.mybir as mybir
import concourse.tile as tile
from concourse import bacc
from concourse.bass_utils import run_bass_kernel_spmd
from bass_rust import add_dep_helper

F32 = mybir.dt.float32
F32R = mybir.dt.float32r
ALU = mybir.AluOpType
PRELU = mybir.ActivationFunctionType.Prelu

B_FULL = 262144
N_CORES = 8
B_CORE = B_FULL // N_CORES          # 32768
N = 512                              # columns per chunk (fp32 PSUM bank)
PIPE = 20                            # 10 stages x 2-step latency
AGE = 2                              # drain-to-consume latency in steps
ALPHA = 0.01                         # leaky-relu slope

# partition row counts
K1 = 50          # t1: a(45) + ilrelu(meta)(4) + ones(1)
K2 = 104         # t2: b(102) + ones(1) + zero pad (even row
                 # count: odd-partition DMAs pin to one queue)
M2 = 69          # bank2 drain: b1(68) + ones(1)
MT = 97          # psT drain partitions (bankE exact; bankO/bank1 padded)

# weight tile column spans (all matmuls M=97 except MM2's 69)
CM1, CM2, CM3, CM4, CM5, CM6 = 0, 97, 166, 263, 360, 457
WT_COLS = 560

NB_IN = 4        # t1/t2 buffer ring depth
NB_ACT = 3       # teo/b1 buffer ring depth


def _ilrelu(x):
    """Inverse of leaky-relu (slope 0.01)."""
    return np.where(x > 0, x, x * (1.0 / ALPHA)).astype(np.float32)


def _pack_weights(Wa, ba, Wb, bb, Ws, Bs):
    """Build the [128, WT_COLS] packed weight tile (biases via ones rows)."""
    W0, W1, W2, W3, W4, W5, W6, W7, W8, W9 = Ws
    B0, B1, B2, B3, B4, B5, B6, B7, B8, B9 = Bs
    wt = np.zeros((128, WT_COLS), np.float32)
    # MM1: rhs t1 -> bank1 [a1(0:68); meta(68:72); ones(72)]
    c = CM1
    wt[0:45, c:c + 68] = Wa
    wt[45:49, c + 68:c + 72] = np.eye(4, dtype=np.float32)
    wt[49, c:c + 68] = ba
    wt[49, c + 72] = 1.0
    # MM2: rhs t2 -> bank2 [b1(0:68); ones(68)]
    c = CM2
    wt[0:102, c:c + 68] = Wb
    wt[102, c:c + 68] = bb
    wt[102, c + 68] = 1.0
    # MM3: rhs T1h -> bankE c0 part (cols 0:34) + ones (col 96)
    c = CM3
    wt[0:68, c:c + 34] = W0[0:68]
    wt[68:72, c:c + 34] = W0[136:140]
    wt[72, c:c + 34] = B0
    wt[72, c + 96] = 1.0
    # MM4: rhs B1 -> bankE c0 part (cols 0:34)
    c = CM4
    wt[0:68, c:c + 34] = W0[68:136]
    # MM5: rhs TOh = [c1;c3;c5;c7;y;ones] -> bankE evens
    c = CM5
    wt[0:34, c + 34:c + 54] = W2    # c1 -> c2
    wt[34:54, c + 54:c + 74] = W4   # c3 -> c4
    wt[54:74, c + 74:c + 94] = W6   # c5 -> c6
    wt[74:79, c + 94:c + 96] = W8   # c7 -> c8
    wt[80, c + 34:c + 54] = B2
    wt[80, c + 54:c + 74] = B4
    wt[80, c + 74:c + 94] = B6
    wt[80, c + 94:c + 96] = B8
    # MM6: rhs TEh = [c0;c2;c4;c6;c8;ones] -> bankO odds
    c = CM6
    wt[0:34, c:c + 34] = W1         # c0 -> c1
    wt[34:54, c + 34:c + 54] = W3   # c2 -> c3
    wt[54:74, c + 54:c + 74] = W5   # c4 -> c5
    wt[74:94, c + 74:c + 79] = W7   # c6 -> c7
    wt[94:96, c + 79:c + 80] = W9   # c8 -> y
    wt[96, c:c + 34] = B1
    wt[96, c + 34:c + 54] = B3
    wt[96, c + 54:c + 74] = B5
    wt[96, c + 74:c + 79] = B7
    wt[96, c + 79] = B9[0]
    wt[96, c + 80] = 1.0
    return wt


def _pack_core_inputs(a, b, meta, n_chunks):
    """Pack one core's shard into the t1/t2 DMA streams."""
    bc = n_chunks * N
    t1 = np.empty((K1, bc), np.float32)
    t1[0:45] = a[:bc].T
    t1[45:49] = _ilrelu(meta[:bc].T)
    t1[49] = 1.0
    t2 = np.zeros((K2, bc), np.float32)
    t2[0:102] = b[:bc].T
    t2[102] = 1.0
    return t1, t2


def build_bass(n_chunks):
    """Build + compile the per-core Bass program (same on all 8 cores)."""
    nc = bacc.Bacc(None, target_bir_lowering=False, debug=False)
    n_steps = n_chunks + PIPE

    t1_d = nc.dram_tensor("t1", [K1, n_chunks * N], F32, kind="ExternalInput")
    t2_d = nc.dram_tensor("t2", [K2, n_chunks * N], F32, kind="ExternalInput")
    wt_d = nc.dram_tensor("wt", [128, WT_COLS], F32, kind="ExternalInput")
    y_d = nc.dram_tensor("y", [1, n_chunks * N], F32, kind="ExternalOutput")

    with tile.TileContext(nc) as tc:
        with (
            tc.tile_pool(name="const", bufs=1) as constp,
            tc.tile_pool(name="psT", bufs=2, space=bass.MemorySpace.PSUM) as psTp,
            tc.tile_pool(name="ps2", bufs=2, space=bass.MemorySpace.PSUM) as ps2p,
        ):
            wt = constp.tile([128, WT_COLS], F32R, tag="wt")
            z1 = constp.tile([128, N], F32R, tag="z1")
            nc.sync.dma_start(wt[:], wt_d[:].bitcast(F32R))
            # all-ones tail substitute: keeps the ones-row bias passthrough
            # alive after the input stream ends; the garbage activations it
            # produces belong to chunks that are never output
            nc.gpsimd.memset(z1[:].bitcast(F32), 1.0)

            # fixed operand buffers, fully zeroed once: writers only touch
            # the live partition rows, so rows above stay 0.0 forever and
            # the K=128 rhs windows multiply clean zeros
            t1b = [constp.tile([128, N], F32R, tag=f"t1b{i}",
                                name=f"t1b{i}") for i in range(NB_IN)]
            t2b = [constp.tile([128, N], F32R, tag=f"t2b{i}",
                                name=f"t2b{i}") for i in range(NB_IN)]
            teob = [constp.tile([128, 3 * N], F32R, tag=f"teob{i}",
                                 name=f"teob{i}") for i in range(NB_ACT)]
            b1b = [constp.tile([128, N], F32R, tag=f"b1b{i}",
                                name=f"b1b{i}") for i in range(NB_ACT)]
            for tl in t1b + t2b + teob + b1b:
                nc.gpsimd.memset(tl[:].bitcast(F32), 0.0)

            def w(c0, m):
                return wt[0:128, c0:c0 + m]

            def chain(*insts):
                for i in range(1, len(insts)):
                    add_dep_helper(insts[i].ins, insts[i - 1].ins,
                                   sync=False, reason="psum acc order")

            def dma_in(c):
                if c < n_chunks:
                    nc.sync.dma_start(
                        t1b[c % NB_IN][0:K1],
                        t1_d[:, c * N:(c + 1) * N].bitcast(F32R))
                    nc.sync.dma_start(
                        t2b[c % NB_IN][0:K2],
                        t2_d[:, c * N:(c + 1) * N].bitcast(F32R))

            dma_in(0)
            dma_in(1)

            for t in range(n_steps):
                dma_in(t + AGE)
                mm = nc.tensor.matmul

                teo = teob[(t - AGE) % NB_ACT]
                b1 = b1b[(t - AGE) % NB_ACT]
                real = t < n_chunks
                rhs1 = t1b[t % NB_IN][0:128] if real else z1[0:128]
                rhs2 = t2b[t % NB_IN][0:128] if real else z1[0:128]

                psT = psTp.tile([128, 3 * N], F32, tag="psT", name=f"psT_{t}")
                ps2 = ps2p.tile([128, N], F32, tag="ps2", name=f"ps2_{t}")

                # ---- chain banks first: their deps are 2 steps old ----
                i1 = mm(psT[0:MT, 0:N], w(CM3, MT), teo[0:128, 2 * N:3 * N],
                        start=True, stop=False, tile_position=(0, 0))
                i2 = mm(psT[0:MT, 0:N], w(CM4, MT), b1[0:128],
                        start=False, stop=False, tile_position=(0, 0))
                i3 = mm(psT[0:MT, 0:N], w(CM5, MT), teo[0:128, N:2 * N],
                        start=False, stop=True, tile_position=(0, 0))
                chain(i1, i2, i3)

                mm(psT[0:MT, N:2 * N], w(CM6, MT), teo[0:128, 0:N],
                   start=True, stop=True, tile_position=(0, 0))

                # ---- stage 1 ----
                mm(psT[0:MT, 2 * N:3 * N], w(CM1, MT), rhs1,
                   start=True, stop=True, tile_position=(0, 0))
                mm(ps2[0:M2], w(CM2, M2), rhs2,
                   start=True, stop=True, tile_position=(0, 0))

                # ---- drains (pure lrelu; biases already in PSUM) ----
                teo_t = teob[t % NB_ACT]
                nc.scalar.activation(teo_t[0:MT, 0:3 * N], psT[0:MT],
                                     PRELU, alpha=ALPHA)
                b1_t = b1b[t % NB_ACT]
                nc.vector.tensor_copy(b1_t[0:M2], ps2[0:M2])
                nc.vector.scalar_tensor_tensor(
                    b1_t[0:M2], b1_t[0:M2], ALPHA, b1_t[0:M2],
                    ALU.mult, ALU.max)

                # ---- y out (row 79 of the odd half) ----
                if t >= PIPE:
                    c = t - PIPE
                    nc.gpsimd.dma_start(
                        y_d[:, c * N:(c + 1) * N].bitcast(F32R),
                        teo_t[79:80, N:2 * N])

    nc.compile()
    return nc


_NC_CACHE = {}


def _get_nc(n_chunks):
    if n_chunks not in _NC_CACHE:
        _NC_CACHE[n_chunks] = build_bass(n_chunks)
    return _NC_CACHE[n_chunks]


def run_cores(inputs, n_chunks, cores, trace=False, trace_kwargs=None):
    """Pack inputs, run the SPMD kernel on the given cores, return
    (per-core y arrays, BassKernelResults)."""
    a = np.asarray(inputs["a"], np.float32)
    b = np.asarray(inputs["b"], np.float32)
    meta = np.asarray(inputs["meta"], np.float32)
    Ws = [np.asarray(inputs[f"W{i}"], np.float32) for i in range(10)]
    Bs = [np.asarray(inputs[f"B{i}"], np.float32) for i in range(10)]
    wt = _pack_weights(np.asarray(inputs["Wa"], np.float32),
                       np.asarray(inputs["ba"], np.float32),
                       np.asarray(inputs["Wb"], np.float32),
                       np.asarray(inputs["bb"], np.float32), Ws, Bs)
    in_maps = []
    for r in cores:
        sl = slice(r * B_CORE, r * B_CORE + n_chunks * N)
        t1, t2 = _pack_core_inputs(a[sl], b[sl], meta[sl], n_chunks)
        in_maps.append({"t1": t1, "t2": t2, "wt": wt})
    nc = _get_nc(n_chunks)
    kw = dict(trace=trace)
    if trace_kwargs:
        kw.update(trace_kwargs)
    res = run_bass_kernel_spmd(nc, in_maps, list(range(len(cores))), **kw)
    return [res.results[i]["y"] for i in range(len(cores))], res


def kernel(**inputs):
    n_chunks = B_CORE // N
    ys, _ = run_cores(inputs, n_chunks, list(range(N_CORES)))
    out = np.empty((B_FULL, 1), np.float32)
    for r in range(N_CORES):
        out[r * B_CORE:(r + 1) * B_CORE, 0] = ys[r][0]
    return out


# revision 11
# speedup vs baseline: 1.8191x; 1.0141x over previous
"""Trainium2 Bass kernel for nn_Net_67954972557347 (dense_mlp).

Network: a1 = lrelu(a@Wa+ba) [B,68]; b1 = lrelu(b@Wb+bb) [B,68];
c = [a1|b1|meta] [B,140]; then 10 lrelu'd dense layers
(140->34->34->20->20->20->20->20->5->2->1), lrelu slope 0.01.

Strategy: pure data parallel over 8 cores (32768 rows each), activations
feature-major ([feat, batch]); batch streams 512 columns per pipeline
step through the PE (fp32r datapath).

6 matmuls / 2 PSUM tiles / 2 drain groups per step:
  psT (3 banks, 1536 cols):
    cols 0:512    bankE [c0; c2; c4; c6; c8; ones] <- MM3(T1h) + MM4(B1)
                                                      + MM5(TOh)
    cols 512:1024 bankO [c1; c3; c5; c7; y; ones]  <- MM6(TEh)
    cols 1024:1536 bank1 [a1; meta; ones]          <- MM1(t1)
  ps2 (1 bank): bank2 [b1; ones]                   <- MM2(t2)
t1 = [a.T; ilrelu(meta); ones], t2 = [b.T; ones] are the DMA streams.
The even/odd chain banks advance all ten tail layers in two
block-diagonal matmuls. ALL biases are folded in-PSUM via ones-row
passthrough columns, so drains are pure leaky-relu: psT drains in ONE
1536-col ACT Prelu into the TEO tile (whose three 512-col halves are
the next step's matmul rhs windows); ps2 drains on DVE (copy +
max(0.01x, x); PSUM cannot be a dual stt operand).

EVERY matmul uses K=128 (full-partition rhs window, zero weight rows
beyond the real contraction) and M>=69: the PE array reconfigures its
tile geometry whenever round-up(K)/round-up(M) changes between
consecutive matmuls, which locks the clock at the mid p-state (0.83
ns/col instead of 0.42 — measured 427 vs 229 ns per 512-col matmul).
Uniform 128x128 tiles keep it at full speed for free (cost scales with
the moving dim only). SBUF operand buffers are fixed, self-managed
rings, fully memset once so the padded partition rows multiply as 0.0
(never NaN garbage).

Latency hiding: matmuls read tiles drained TWO steps ago (age-2) and
input DMA is prefetched two steps ahead, so the PE's in-order queue
never waits on same-step drains. The t2 stream is padded to an even
partition count: odd-partition DMAs land on a single DMA queue instead
of spreading across all 16. Pipeline depth 2 steps/layer * 10 stages
= 20 steps.
"""

import os
import sys

import numpy as np

for _p in ("/opt/trn_rl_repo", "/root/.axon_site/_ro/trn_rl_repo"):
    if os.path.isdir(_p) and _p not in sys.path:
        sys.path.append(_p)

import concourse.bass as bass
import concourse.mybir as mybir
import concourse.tile as tile
from concourse import bacc
from concourse.bass_utils import run_bass_kernel_spmd
from bass_rust import add_dep_helper

F32 = mybir.dt.float32
F32R = mybir.dt.float32r
ALU = mybir.AluOpType
PRELU = mybir.ActivationFunctionType.Prelu

B_FULL = 262144
N_CORES = 8
B_CORE = B_FULL // N_CORES          # 32768
N = 512                              # columns per chunk (fp32 PSUM bank)
PIPE = 20                            # 10 stages x 2-step latency
AGE = 2                              # drain-to-consume latency in steps
ALPHA = 0.01                         # leaky-relu slope

# partition row counts
K1 = 50          # t1: a(45) + ilrelu(meta)(4) + ones(1)
K2 = 104         # t2: b(102) + ones(1) + zero pad (even row
                 # count: odd-partition DMAs pin to one queue)
M2 = 69          # bank2 drain: b1(68) + ones(1)
MT = 97          # psT drain partitions (bankE exact; bankO/bank1 padded)

# weight tile column spans (all matmuls M=97 except MM2's 69)
CM1, CM2, CM3, CM4, CM5, CM6 = 0, 97, 166, 263, 360, 457
WT_COLS = 560

NB_IN = 6        # t1/t2 buffer ring depth
NB_ACT = 4       # teo/b1 buffer ring depth


def _ilrelu(x):
    """Inverse of leaky-relu (slope 0.01)."""
    return np.where(x > 0, x, x * (1.0 / ALPHA)).astype(np.float32)


def _pack_weights(Wa, ba, Wb, bb, Ws, Bs):
    """Build the [128, WT_COLS] packed weight tile (biases via ones rows)."""
    W0, W1, W2, W3, W4, W5, W6, W7, W8, W9 = Ws
    B0, B1, B2, B3, B4, B5, B6, B7, B8, B9 = Bs
    wt = np.zeros((128, WT_COLS), np.float32)
    # MM1: rhs t1 -> bank1 [a1(0:68); meta(68:72); ones(72)]
    c = CM1
    wt[0:45, c:c + 68] = Wa
    wt[45:49, c + 68:c + 72] = np.eye(4, dtype=np.float32)
    wt[49, c:c + 68] = ba
    wt[49, c + 72] = 1.0
    # MM2: rhs t2 -> bank2 [b1(0:68); ones(68)]
    c = CM2
    wt[0:102, c:c + 68] = Wb
    wt[102, c:c + 68] = bb
    wt[102, c + 68] = 1.0
    # MM3: rhs T1h -> bankE c0 part (cols 0:34) + ones (col 96)
    c = CM3
    wt[0:68, c:c + 34] = W0[0:68]
    wt[68:72, c:c + 34] = W0[136:140]
    wt[72, c:c + 34] = B0
    wt[72, c + 96] = 1.0
    # MM4: rhs B1 -> bankE c0 part (cols 0:34)
    c = CM4
    wt[0:68, c:c + 34] = W0[68:136]
    # MM5: rhs TOh = [c1;c3;c5;c7;y;ones] -> bankE evens
    c = CM5
    wt[0:34, c + 34:c + 54] = W2    # c1 -> c2
    wt[34:54, c + 54:c + 74] = W4   # c3 -> c4
    wt[54:74, c + 74:c + 94] = W6   # c5 -> c6
    wt[74:79, c + 94:c + 96] = W8   # c7 -> c8
    wt[80, c + 34:c + 54] = B2
    wt[80, c + 54:c + 74] = B4
    wt[80, c + 74:c + 94] = B6
    wt[80, c + 94:c + 96] = B8
    # MM6: rhs TEh = [c0;c2;c4;c6;c8;ones] -> bankO odds
    c = CM6
    wt[0:34, c:c + 34] = W1         # c0 -> c1
    wt[34:54, c + 34:c + 54] = W3   # c2 -> c3
    wt[54:74, c + 54:c + 74] = W5   # c4 -> c5
    wt[74:94, c + 74:c + 79] = W7   # c6 -> c7
    wt[94:96, c + 79:c + 80] = W9   # c8 -> y
    wt[96, c:c + 34] = B1
    wt[96, c + 34:c + 54] = B3
    wt[96, c + 54:c + 74] = B5
    wt[96, c + 74:c + 79] = B7
    wt[96, c + 79] = B9[0]
    wt[96, c + 80] = 1.0
    return wt


def _pack_core_inputs(a, b, meta, n_chunks):
    """Pack one core's shard into the t1/t2 DMA streams."""
    bc = n_chunks * N
    t1 = np.empty((K1, bc), np.float32)
    t1[0:45] = a[:bc].T
    t1[45:49] = _ilrelu(meta[:bc].T)
    t1[49] = 1.0
    t2 = np.zeros((K2, bc), np.float32)
    t2[0:102] = b[:bc].T
    t2[102] = 1.0
    return t1, t2


def build_bass(n_chunks):
    """Build + compile the per-core Bass program (same on all 8 cores)."""
    nc = bacc.Bacc(None, target_bir_lowering=False, debug=False)
    n_steps = n_chunks + PIPE

    t1_d = nc.dram_tensor("t1", [K1, n_chunks * N], F32, kind="ExternalInput")
    t2_d = nc.dram_tensor("t2", [K2, n_chunks * N], F32, kind="ExternalInput")
    wt_d = nc.dram_tensor("wt", [128, WT_COLS], F32, kind="ExternalInput")
    y_d = nc.dram_tensor("y", [1, n_chunks * N], F32, kind="ExternalOutput")

    with tile.TileContext(nc) as tc:
        with (
            tc.tile_pool(name="const", bufs=1) as constp,
            tc.tile_pool(name="psT", bufs=2, space=bass.MemorySpace.PSUM) as psTp,
            tc.tile_pool(name="ps2", bufs=2, space=bass.MemorySpace.PSUM) as ps2p,
        ):
            wt = constp.tile([128, WT_COLS], F32R, tag="wt")
            nc.sync.dma_start(wt[:], wt_d[:].bitcast(F32R))

            # fixed operand buffers, fully zeroed once: writers only touch
            # the live partition rows, so rows above stay 0.0 forever and
            # the K=128 rhs windows multiply clean zeros
            t1b = [constp.tile([128, N], F32R, tag=f"t1b{i}",
                                name=f"t1b{i}") for i in range(NB_IN)]
            t2b = [constp.tile([128, N], F32R, tag=f"t2b{i}",
                                name=f"t2b{i}") for i in range(NB_IN)]
            teob = [constp.tile([128, 3 * N], F32R, tag=f"teob{i}",
                                 name=f"teob{i}") for i in range(NB_ACT)]
            b1b = [constp.tile([128, N], F32R, tag=f"b1b{i}",
                                name=f"b1b{i}") for i in range(NB_ACT)]
            for tl in t1b + t2b + teob + b1b:
                nc.gpsimd.memset(tl[:].bitcast(F32), 0.0)

            def w(c0, m):
                return wt[0:128, c0:c0 + m]

            def chain(*insts):
                for i in range(1, len(insts)):
                    add_dep_helper(insts[i].ins, insts[i - 1].ins,
                                   sync=False, reason="psum acc order")

            def dma_in(c):
                if c < n_chunks:
                    nc.sync.dma_start(
                        t1b[c % NB_IN][0:K1],
                        t1_d[:, c * N:(c + 1) * N].bitcast(F32R))
                    nc.sync.dma_start(
                        t2b[c % NB_IN][0:K2],
                        t2_d[:, c * N:(c + 1) * N].bitcast(F32R))

            for c in range(AGE + 2):
                dma_in(c)

            for t in range(n_steps):
                dma_in(t + AGE + 2)
                mm = nc.tensor.matmul

                teo = teob[(t - AGE) % NB_ACT]
                b1 = b1b[(t - AGE) % NB_ACT]
                # tail steps (t >= n_chunks) skip stage 1 entirely: the
                # chain then consumes STALE bank1/bank2 drains (finite, and
                # their ones rows still carry the bias passthrough); those
                # chunks are never output
                real = t < n_chunks

                psT = psTp.tile([128, 3 * N], F32, tag="psT", name=f"psT_{t}")
                ps2 = ps2p.tile([128, N], F32, tag="ps2", name=f"ps2_{t}")

                # ---- chain banks first: their deps are 2 steps old ----
                i1 = mm(psT[0:MT, 0:N], w(CM3, MT), teo[0:128, 2 * N:3 * N],
                        start=True, stop=False, tile_position=(0, 0))
                i2 = mm(psT[0:MT, 0:N], w(CM4, MT), b1[0:128],
                        start=False, stop=False, tile_position=(0, 0))
                i3 = mm(psT[0:MT, 0:N], w(CM5, MT), teo[0:128, N:2 * N],
                        start=False, stop=True, tile_position=(0, 0))
                chain(i1, i2, i3)

                mm(psT[0:MT, N:2 * N], w(CM6, MT), teo[0:128, 0:N],
                   start=True, stop=True, tile_position=(0, 0))

                # ---- stage 1 ----
                if real:
                    mm(psT[0:MT, 2 * N:3 * N], w(CM1, MT),
                       t1b[t % NB_IN][0:128],
                       start=True, stop=True, tile_position=(0, 0))
                    mm(ps2[0:M2], w(CM2, M2), t2b[t % NB_IN][0:128],
                       start=True, stop=True, tile_position=(0, 0))

                # ---- drains (pure lrelu; biases already in PSUM) ----
                teo_t = teob[t % NB_ACT]
                nc.scalar.activation(teo_t[0:MT, 0:3 * N], psT[0:MT],
                                     PRELU, alpha=ALPHA)
                if real:
                    b1_t = b1b[t % NB_ACT]
                    nc.vector.tensor_copy(b1_t[0:M2], ps2[0:M2])
                    nc.vector.scalar_tensor_tensor(
                        b1_t[0:M2], b1_t[0:M2], ALPHA, b1_t[0:M2],
                        ALU.mult, ALU.max)

                # ---- y out (row 79 of the odd half) ----
                if t >= PIPE:
                    c = t - PIPE
                    nc.gpsimd.dma_start(
                        y_d[:, c * N:(c + 1) * N].bitcast(F32R),
                        teo_t[79:80, N:2 * N])

    nc.compile()
    return nc


_NC_CACHE = {}


def _get_nc(n_chunks):
    if n_chunks not in _NC_CACHE:
        _NC_CACHE[n_chunks] = build_bass(n_chunks)
    return _NC_CACHE[n_chunks]


def run_cores(inputs, n_chunks, cores, trace=False, trace_kwargs=None):
    """Pack inputs, run the SPMD kernel on the given cores, return
    (per-core y arrays, BassKernelResults)."""
    a = np.asarray(inputs["a"], np.float32)
    b = np.asarray(inputs["b"], np.float32)
    meta = np.asarray(inputs["meta"], np.float32)
    Ws = [np.asarray(inputs[f"W{i}"], np.float32) for i in range(10)]
    Bs = [np.asarray(inputs[f"B{i}"], np.float32) for i in range(10)]
    wt = _pack_weights(np.asarray(inputs["Wa"], np.float32),
                       np.asarray(inputs["ba"], np.float32),
                       np.asarray(inputs["Wb"], np.float32),
                       np.asarray(inputs["bb"], np.float32), Ws, Bs)
    in_maps = []
    for r in cores:
        sl = slice(r * B_CORE, r * B_CORE + n_chunks * N)
        t1, t2 = _pack_core_inputs(a[sl], b[sl], meta[sl], n_chunks)
        in_maps.append({"t1": t1, "t2": t2, "wt": wt})
    nc = _get_nc(n_chunks)
    kw = dict(trace=trace)
    if trace_kwargs:
        kw.update(trace_kwargs)
    res = run_bass_kernel_spmd(nc, in_maps, list(range(len(cores))), **kw)
    return [res.results[i]["y"] for i in range(len(cores))], res


def kernel(**inputs):
    n_chunks = B_CORE // N
    ys, _ = run_cores(inputs, n_chunks, list(range(N_CORES)))
    out = np.empty((B_FULL, 1), np.float32)
    for r in range(N_CORES):
        out[r * B_CORE:(r + 1) * B_CORE, 0] = ys[r][0]
    return out


# revision 12
# speedup vs baseline: 1.8543x; 1.0194x over previous
"""Trainium2 Bass kernel for nn_Net_67954972557347 (dense_mlp).

Network: a1 = lrelu(a@Wa+ba) [B,68]; b1 = lrelu(b@Wb+bb) [B,68];
c = [a1|b1|meta] [B,140]; then 10 lrelu'd dense layers
(140->34->34->20->20->20->20->20->5->2->1), lrelu slope 0.01.

Strategy: pure data parallel over 8 cores (32768 rows each), activations
feature-major ([feat, batch]); batch streams 512 columns per pipeline
step through the PE (fp32r datapath).

6 matmuls / 2 PSUM tiles / 2 drain groups per step:
  psT (3 banks, 1536 cols):
    cols 0:512    bankE [c0; c2; c4; c6; c8; ones] <- MM3(T1h) + MM4(B1)
                                                      + MM5(TOh)
    cols 512:1024 bankO [c1; c3; c5; c7; y; ones]  <- MM6(TEh)
    cols 1024:1536 bank1 [a1; meta; ones]          <- MM1(t1)
  ps2 (1 bank): bank2 [b1; ones]                   <- MM2(t2)
t1 = [a.T; ilrelu(meta); ones], t2 = [b.T; ones] are the DMA streams.
The even/odd chain banks advance all ten tail layers in two
block-diagonal matmuls. ALL biases are folded in-PSUM via ones-row
passthrough columns, so drains are pure leaky-relu: psT drains in ONE
1536-col ACT Prelu into the TEO tile (whose three 512-col halves are
the next step's matmul rhs windows); ps2 drains on DVE (copy +
max(0.01x, x); PSUM cannot be a dual stt operand).

EVERY matmul uses K=128 (full-partition rhs window, zero weight rows
beyond the real contraction) and M>=69: the PE array reconfigures its
tile geometry whenever round-up(K)/round-up(M) changes between
consecutive matmuls, which locks the clock at the mid p-state (0.83
ns/col instead of 0.42 — measured 427 vs 229 ns per 512-col matmul).
Uniform 128x128 tiles keep it at full speed for free (cost scales with
the moving dim only). SBUF operand buffers are fixed, self-managed
rings, fully memset once so the padded partition rows multiply as 0.0
(never NaN garbage).

Latency hiding: matmuls read tiles drained TWO steps ago (age-2) and
input DMA is prefetched two steps ahead, so the PE's in-order queue
never waits on same-step drains. The t2 stream is padded to an even
partition count: odd-partition DMAs land on a single DMA queue instead
of spreading across all 16. Pipeline depth 2 steps/layer * 10 stages
= 20 steps.
"""

import os
import sys

import numpy as np

for _p in ("/opt/trn_rl_repo", "/root/.axon_site/_ro/trn_rl_repo"):
    if os.path.isdir(_p) and _p not in sys.path:
        sys.path.append(_p)

import concourse.bass as bass
import concourse.mybir as mybir
import concourse.tile as tile
from concourse import bacc
from concourse.bass_utils import run_bass_kernel_spmd
from bass_rust import add_dep_helper

F32 = mybir.dt.float32
F32R = mybir.dt.float32r
ALU = mybir.AluOpType
PRELU = mybir.ActivationFunctionType.Prelu

B_FULL = 262144
N_CORES = 8
B_CORE = B_FULL // N_CORES          # 32768
N = 512                              # columns per chunk (fp32 PSUM bank)
PIPE = 20                            # 10 stages x 2-step latency
AGE = 2                              # drain-to-consume latency in steps
ALPHA = 0.01                         # leaky-relu slope

# partition row counts
K1 = 50          # t1: a(45) + ilrelu(meta)(4) + ones(1)
K2 = 104         # t2: b(102) + ones(1) + zero pad (even row
                 # count: odd-partition DMAs pin to one queue)
MT = 128         # matmul M / drain partitions: padded to the full 128 so
                 # every drain also rewrites the pad rows with lrelu(0)=0 —
                 # no startup memset needed for the activation buffers
M2 = MT

# weight tile column spans (every matmul M=128)
CM1, CM2, CM3, CM4, CM5, CM6 = 0, 128, 256, 384, 512, 640
WT_COLS = 768

NB_IN = 6        # t1/t2 buffer ring depth
NB_ACT = 4       # teo/b1 buffer ring depth


def _ilrelu(x):
    """Inverse of leaky-relu (slope 0.01)."""
    return np.where(x > 0, x, x * (1.0 / ALPHA)).astype(np.float32)


def _pack_weights(Wa, ba, Wb, bb, Ws, Bs):
    """Build the [128, WT_COLS] packed weight tile (biases via ones rows)."""
    W0, W1, W2, W3, W4, W5, W6, W7, W8, W9 = Ws
    B0, B1, B2, B3, B4, B5, B6, B7, B8, B9 = Bs
    wt = np.zeros((128, WT_COLS), np.float32)
    # MM1: rhs t1 -> bank1 [a1(0:68); meta(68:72); ones(72)]
    c = CM1
    wt[0:45, c:c + 68] = Wa
    wt[45:49, c + 68:c + 72] = np.eye(4, dtype=np.float32)
    wt[49, c:c + 68] = ba
    wt[49, c + 72] = 1.0
    # MM2: rhs t2 -> bank2 [b1(0:68); ones(68)]
    c = CM2
    wt[0:102, c:c + 68] = Wb
    wt[102, c:c + 68] = bb
    wt[102, c + 68] = 1.0
    # MM3: rhs T1h -> bankE c0 part (cols 0:34) + ones (col 96)
    c = CM3
    wt[0:68, c:c + 34] = W0[0:68]
    wt[68:72, c:c + 34] = W0[136:140]
    wt[72, c:c + 34] = B0
    wt[72, c + 96] = 1.0
    # MM4: rhs B1 -> bankE c0 part (cols 0:34)
    c = CM4
    wt[0:68, c:c + 34] = W0[68:136]
    # MM5: rhs TOh = [c1;c3;c5;c7;y;ones] -> bankE evens
    c = CM5
    wt[0:34, c + 34:c + 54] = W2    # c1 -> c2
    wt[34:54, c + 54:c + 74] = W4   # c3 -> c4
    wt[54:74, c + 74:c + 94] = W6   # c5 -> c6
    wt[74:79, c + 94:c + 96] = W8   # c7 -> c8
    wt[80, c + 34:c + 54] = B2
    wt[80, c + 54:c + 74] = B4
    wt[80, c + 74:c + 94] = B6
    wt[80, c + 94:c + 96] = B8
    # MM6: rhs TEh = [c0;c2;c4;c6;c8;ones] -> bankO odds
    c = CM6
    wt[0:34, c:c + 34] = W1         # c0 -> c1
    wt[34:54, c + 34:c + 54] = W3   # c2 -> c3
    wt[54:74, c + 54:c + 74] = W5   # c4 -> c5
    wt[74:94, c + 74:c + 79] = W7   # c6 -> c7
    wt[94:96, c + 79:c + 80] = W9   # c8 -> y
    wt[96, c:c + 34] = B1
    wt[96, c + 34:c + 54] = B3
    wt[96, c + 54:c + 74] = B5
    wt[96, c + 74:c + 79] = B7
    wt[96, c + 79] = B9[0]
    wt[96, c + 80] = 1.0
    return wt


def _pack_core_inputs(a, b, meta, n_chunks):
    """Pack one core's shard into the t1/t2 DMA streams."""
    bc = n_chunks * N
    t1 = np.empty((K1, bc), np.float32)
    t1[0:45] = a[:bc].T
    t1[45:49] = _ilrelu(meta[:bc].T)
    t1[49] = 1.0
    t2 = np.zeros((K2, bc), np.float32)
    t2[0:102] = b[:bc].T
    t2[102] = 1.0
    return t1, t2


def build_bass(n_chunks):
    """Build + compile the per-core Bass program (same on all 8 cores)."""
    nc = bacc.Bacc(None, target_bir_lowering=False, debug=False)
    n_steps = n_chunks + PIPE

    t1_d = nc.dram_tensor("t1", [K1, n_chunks * N], F32, kind="ExternalInput")
    t2_d = nc.dram_tensor("t2", [K2, n_chunks * N], F32, kind="ExternalInput")
    wt_d = nc.dram_tensor("wt", [128, WT_COLS], F32, kind="ExternalInput")
    y_d = nc.dram_tensor("y", [1, n_chunks * N], F32, kind="ExternalOutput")

    with tile.TileContext(nc) as tc:
        with (
            tc.tile_pool(name="const", bufs=1) as constp,
            tc.tile_pool(name="psT", bufs=2, space=bass.MemorySpace.PSUM) as psTp,
            tc.tile_pool(name="ps2", bufs=2, space=bass.MemorySpace.PSUM) as ps2p,
        ):
            wt = constp.tile([128, WT_COLS], F32R, tag="wt")
            nc.sync.dma_start(wt[:], wt_d[:].bitcast(F32R))

            # fixed operand buffers, fully zeroed once: writers only touch
            # the live partition rows, so rows above stay 0.0 forever and
            # the K=128 rhs windows multiply clean zeros
            t1b = [constp.tile([128, N], F32R, tag=f"t1b{i}",
                                name=f"t1b{i}") for i in range(NB_IN)]
            t2b = [constp.tile([128, N], F32R, tag=f"t2b{i}",
                                name=f"t2b{i}") for i in range(NB_IN)]
            teob = [constp.tile([128, 3 * N], F32R, tag=f"teob{i}",
                                 name=f"teob{i}") for i in range(NB_ACT)]
            b1b = [constp.tile([128, N], F32R, tag=f"b1b{i}",
                                name=f"b1b{i}") for i in range(NB_ACT)]
            # teob/b1b need no memset: the [0:128] drains rewrite every row
            # each step, and the first consumers read age<0 generations only
            # after those buffers were drained at least once... the very
            # first two steps DO read undrained teob/b1b buffers, so zero
            # them; split the memsets across two engines to halve startup.
            for i, tl in enumerate(t1b + t2b + teob + b1b):
                eng = nc.gpsimd if i % 2 == 0 else nc.vector
                eng.memset(tl[:].bitcast(F32), 0.0)

            def w(c0, m):
                return wt[0:128, c0:c0 + m]

            def chain(*insts):
                for i in range(1, len(insts)):
                    add_dep_helper(insts[i].ins, insts[i - 1].ins,
                                   sync=False, reason="psum acc order")

            def dma_in(c):
                if c < n_chunks:
                    nc.sync.dma_start(
                        t1b[c % NB_IN][0:K1],
                        t1_d[:, c * N:(c + 1) * N].bitcast(F32R))
                    nc.sync.dma_start(
                        t2b[c % NB_IN][0:K2],
                        t2_d[:, c * N:(c + 1) * N].bitcast(F32R))

            for c in range(AGE + 2):
                dma_in(c)

            for t in range(n_steps):
                dma_in(t + AGE + 2)
                mm = nc.tensor.matmul

                teo = teob[(t - AGE) % NB_ACT]
                b1 = b1b[(t - AGE) % NB_ACT]
                # tail steps (t >= n_chunks) skip stage 1 entirely: the
                # chain then consumes STALE bank1/bank2 drains (finite, and
                # their ones rows still carry the bias passthrough); those
                # chunks are never output
                real = t < n_chunks

                psT = psTp.tile([128, 3 * N], F32, tag="psT", name=f"psT_{t}")
                ps2 = ps2p.tile([128, N], F32, tag="ps2", name=f"ps2_{t}")

                # ---- chain banks first: their deps are 2 steps old ----
                i1 = mm(psT[0:MT, 0:N], w(CM3, MT), teo[0:128, 2 * N:3 * N],
                        start=True, stop=False, tile_position=(0, 0))
                i2 = mm(psT[0:MT, 0:N], w(CM4, MT), b1[0:128],
                        start=False, stop=False, tile_position=(0, 0))
                i3 = mm(psT[0:MT, 0:N], w(CM5, MT), teo[0:128, N:2 * N],
                        start=False, stop=True, tile_position=(0, 0))

                chain(i1, i2, i3)

                mm(psT[0:MT, N:2 * N], w(CM6, MT), teo[0:128, 0:N],
                   start=True, stop=True, tile_position=(0, 0))

                # ---- stage 1 ----
                if real:
                    mm(psT[0:MT, 2 * N:3 * N], w(CM1, MT),
                       t1b[t % NB_IN][0:128],
                       start=True, stop=True, tile_position=(0, 0))
                    mm(ps2[0:M2], w(CM2, M2), t2b[t % NB_IN][0:128],
                       start=True, stop=True, tile_position=(0, 0))

                # ---- drains (pure lrelu; biases already in PSUM) ----
                teo_t = teob[t % NB_ACT]
                nc.scalar.activation(teo_t[0:MT, 0:3 * N], psT[0:MT],
                                     PRELU, alpha=ALPHA)
                if real:
                    b1_t = b1b[t % NB_ACT]
                    nc.vector.tensor_copy(b1_t[0:M2], ps2[0:M2])
                    nc.vector.scalar_tensor_tensor(
                        b1_t[0:M2], b1_t[0:M2], ALPHA, b1_t[0:M2],
                        ALU.mult, ALU.max)

                # ---- y out (row 79 of the odd half) ----
                if t >= PIPE:
                    c = t - PIPE
                    nc.gpsimd.dma_start(
                        y_d[:, c * N:(c + 1) * N].bitcast(F32R),
                        teo_t[79:80, N:2 * N])

    nc.compile()
    return nc


_NC_CACHE = {}


def _get_nc(n_chunks):
    if n_chunks not in _NC_CACHE:
        _NC_CACHE[n_chunks] = build_bass(n_chunks)
    return _NC_CACHE[n_chunks]


def run_cores(inputs, n_chunks, cores, trace=False, trace_kwargs=None):
    """Pack inputs, run the SPMD kernel on the given cores, return
    (per-core y arrays, BassKernelResults)."""
    a = np.asarray(inputs["a"], np.float32)
    b = np.asarray(inputs["b"], np.float32)
    meta = np.asarray(inputs["meta"], np.float32)
    Ws = [np.asarray(inputs[f"W{i}"], np.float32) for i in range(10)]
    Bs = [np.asarray(inputs[f"B{i}"], np.float32) for i in range(10)]
    wt = _pack_weights(np.asarray(inputs["Wa"], np.float32),
                       np.asarray(inputs["ba"], np.float32),
                       np.asarray(inputs["Wb"], np.float32),
                       np.asarray(inputs["bb"], np.float32), Ws, Bs)
    in_maps = []
    for r in cores:
        sl = slice(r * B_CORE, r * B_CORE + n_chunks * N)
        t1, t2 = _pack_core_inputs(a[sl], b[sl], meta[sl], n_chunks)
        in_maps.append({"t1": t1, "t2": t2, "wt": wt})
    nc = _get_nc(n_chunks)
    kw = dict(trace=trace)
    if trace_kwargs:
        kw.update(trace_kwargs)
    res = run_bass_kernel_spmd(nc, in_maps, list(range(len(cores))), **kw)
    return [res.results[i]["y"] for i in range(len(cores))], res


def kernel(**inputs):
    n_chunks = B_CORE // N
    ys, _ = run_cores(inputs, n_chunks, list(range(N_CORES)))
    out = np.empty((B_FULL, 1), np.float32)
    for r in range(N_CORES):
        out[r * B_CORE:(r + 1) * B_CORE, 0] = ys[r][0]
    return out
